# revision 36
# baseline (speedup 1.0000x reference)
"""Trainium2 Bass kernel for a dense transformer block (prefill), SPMD on 8 NeuronCores.

Sharding strategy:
  - Attention: tensor-parallel over heads (16 heads -> 2 per core), both batches
    on every core. Each core computes RMSNorm(x) -> qkv for its heads -> causal
    attention -> output-projection partial (row-parallel w_proj slice).
  - Proj partials are ReduceScatter-summed over the 8 cores (one RS per batch,
    scattered over the token dim), so core c ends up owning 256 tokens of each
    batch (512 total).
  - FFN: token-parallel. Each core runs residual + RMSNorm2 + gate/up/down over
    its 512 tokens with the full FF weights (streamed from HBM).
  - Host side only does layout work: transposes/slices inputs, concatenates
    output shards.

Matmuls run in bf16 (k/v cache values and all accumulations stay fp32;
the two ReduceScatters per batch move bf16 partials).
"""

import numpy as np
from contextlib import ExitStack

import concourse.bacc as bacc
import concourse.bass as bass
import concourse.tile as tile
import concourse.mybir as mybir
from concourse.bass_utils import run_bass_kernel_spmd
from concourse.masks import make_identity

F32R = mybir.dt.float32r
F32 = mybir.dt.float32
BF16 = mybir.dt.bfloat16
AF = mybir.ActivationFunctionType
ALU = mybir.AluOpType
ts = bass.ts

B, T, D = 2, 2048, 1024
H, DH = 16, 64
NC = 8
HPC = H // NC            # heads per core = 2
HD = HPC * DH            # 128 head-dims per core
KT = D // 128            # 8 k-tiles over D
TCH = T // 512           # 4 query chunks per batch
TBLK = T // 128          # 16 token tiles per batch
SHB = T // NC            # 256 tokens per (core, batch) after RS
SH = B * SHB             # 512 tokens per core shard
DFF = 4096
FFC = DFF // 512         # 8 ff chunks
FFT = DFF // 128         # 32 ff tiles
EPS = 1e-6
SCALE = 1.0 / np.sqrt(DH)

_BUILT = {}


def _emit(nc, io):
    xt, xs, wq, wk, wv, wp, w1, w2, wg, wu, wd = (
        io["xt"], io["xs"], io["wq_t"], io["wk_t"], io["wv_t"], io["wp_t"],
        io["w1r"], io["w2"], io["wg_t"], io["wu_t"], io["wd_t"],
    )
    xo, ko, vo = io["xo"], io["ko"], io["vo"]
    cc_in = [io["cc_in0"], io["cc_in1"]]
    cc_out = [
        [io[f"cc_out{b}{hf}"] for hf in range(2)] for b in range(B)
    ]

    with tile.TileContext(nc) as tc, ExitStack() as ctx:
        ctx.enter_context(
            nc.allow_low_precision(reason="bf16/fp32r intermediates for PE matmuls")
        )
        const = ctx.enter_context(tc.tile_pool(name="const", bufs=1))
        trps = ctx.enter_context(tc.tile_pool(name="trps", bufs=2, space="PSUM"))
        cache_pool = ctx.enter_context(tc.tile_pool(name="cache", bufs=2))

        ident_f = const.tile([128, 128], F32)
        make_identity(nc, ident_f)
        ident = const.tile([128, 128], F32R)
        nc.vector.tensor_copy(out=ident, in_=ident_f)
        eps128 = const.tile([128, 1], F32)
        nc.vector.memset(eps128, EPS)
        w2_sb = const.tile([128, D], F32)
        nc.sync.dma_start(
            out=w2_sb,
            in_=bass.AP(tensor=w2.tensor, offset=0, ap=[[0, 128], [1, D]]),
        )

        kT_cache = []

        # =====================  attention era  ==============================
        with ExitStack() as actx:
            catt = actx.enter_context(tc.tile_pool(name="catt", bufs=1))
            ones_col = catt.tile([128, 128], BF16)
            nc.vector.memset(ones_col, 1.0)
            vinit_col = catt.tile([128, TBLK], F32)
            nc.vector.memset(vinit_col, 1.0)
            mask_b = catt.tile([128, 4, 512], BF16)
            eps1 = catt.tile([1, 1], F32)
            nc.vector.memset(eps1, EPS)
            w1_sb = catt.tile([128, KT], F32)
            nc.sync.dma_start(out=w1_sb, in_=w1)
            wq_sb = catt.tile([128, KT, HD], BF16)
            nc.sync.dma_start(out=wq_sb, in_=wq.rearrange("(k p) m -> p k m", p=128))
            wk_sb = catt.tile([128, KT, HD], BF16)
            nc.sync.dma_start(out=wk_sb, in_=wk.rearrange("(k p) m -> p k m", p=128))
            wv_sb = catt.tile([128, KT, HD], BF16)
            nc.sync.dma_start(out=wv_sb, in_=wv.rearrange("(k p) m -> p k m", p=128))
            wp_sb = catt.tile([128, D], BF16)
            nc.sync.dma_start(out=wp_sb, in_=wp)

            small = actx.enter_context(tc.tile_pool(name="small", bufs=2))
            rsb_pool = actx.enter_context(tc.tile_pool(name="rsb", bufs=2))
            ht_pool = actx.enter_context(tc.tile_pool(name="ht", bufs=KT))

            # ---- phase 0: RMSNorm1 stats (both batches) ----
            rs_bs = []
            xk_b0 = []
            def emit_phase01(b, ss_ps, scratch):
                ss = [
                    ss_ps.tile([128, 512], F32, tag="ssq", name=f"ss{b}_{i}")
                    for i in range(TCH)
                ]
                rsb = rsb_pool.tile([128, T], F32, tag="rsb", name=f"rsb{b}")
                rs_bs.append(rsb)
                xk = [
                    ht_pool.tile([128, T], BF16, tag="ht", name=f"ht{b}_{k}")
                    for k in range(KT)
                ]
                for ch in range(TCH):
                    for k in range(KT):
                        nc.sync.dma_start(
                            out=xk[k][:, ts(ch, 512)],
                            in_=xt[b, ts(k, 128), ts(ch, 512)],
                        )
                        sq = scratch.tile([128, 512], BF16, tag="sqs")
                        nc.vector.tensor_mul(sq, xk[k][:, ts(ch, 512)],
                                             xk[k][:, ts(ch, 512)])
                        nc.tensor.matmul(
                            ss[ch], ones_col, sq,
                            start=(k == 0), stop=(k == KT - 1),
                        )
                    srt = small.tile([1, 512], F32, tag="sc")
                    nc.scalar.activation(srt, ss[ch][0:1, :], AF.Sqrt, bias=eps1,
                                         scale=1.0 / D)
                    nc.gpsimd.partition_broadcast(rsb[:, ts(ch, 512)], srt)
                    nc.vector.reciprocal_approx_fast(
                        out=rsb[:, ts(ch, 512)], in_=rsb[:, ts(ch, 512)]
                    )
                    for k in range(KT):
                        nc.vector.scalar_tensor_tensor(
                            out=xk[k][:, ts(ch, 512)],
                            in0=xk[k][:, ts(ch, 512)],
                            scalar=w1_sb[:, k : k + 1],
                            in1=rsb[:, ts(ch, 512)],
                            op0=ALU.mult, op1=ALU.mult,
                        )
                return xk

            qkv_sb = actx.enter_context(tc.tile_pool(name="qkvsb", bufs=2))
            vaug_pool = actx.enter_context(tc.tile_pool(name="vaug", bufs=4))
            out2_pool = actx.enter_context(tc.tile_pool(name="out2", bufs=2))
            exp_pool = actx.enter_context(tc.tile_pool(name="exp", bufs=4))
            rb_pool = actx.enter_context(tc.tile_pool(name="rb", bufs=1))
            kv_stage = actx.enter_context(tc.tile_pool(name="kvst", bufs=2))
            pr_stage = actx.enter_context(tc.tile_pool(name="prst", bufs=2))

            def emit_qkv(b, ht):
                qp = [
                    qkv_sb.tile([128, T], BF16, tag=f"qp{h}", name=f"qp{b}_{h}")
                    for h in range(HPC)
                ]
                kp = [
                    qkv_sb.tile([128, T], BF16, tag=f"kp{h}", name=f"kp{b}_{h}")
                    for h in range(HPC)
                ]
                for h in range(HPC):
                    nc.gpsimd.memset(qp[h][ts(1 - h, DH), :], 0.0)
                    nc.gpsimd.memset(kp[h][ts(1 - h, DH), :], 0.0)
                kT = cache_pool.tile([128, T], F32R, tag="kT", name=f"kT{b}")
                kT_cache.append(kT)
                vT = qkv_sb.tile([128, T], F32R, tag="vT", name=f"vT{b}")
                cp0 = nc.scalar.copy if b == 0 else (
                    lambda out, in_: nc.vector.tensor_copy(out=out, in_=in_)
                )
                for ch in range(TCH):
                    q_ps = qkv_ps.tile([128, 512], F32, tag="qkvps")
                    k_ps = qkv_ps.tile([128, 512], F32, tag="qkvps")
                    v_ps = qkv_ps.tile([128, 512], F32, tag="qkvps")
                    for k in range(KT):
                        st = (k == 0)
                        sp = (k == KT - 1)
                        rhs = ht[k][:, ts(ch, 512)]
                        nc.tensor.matmul(q_ps, wq_sb[:, k, :], rhs, start=st, stop=sp)
                        nc.tensor.matmul(k_ps, wk_sb[:, k, :], rhs, start=st, stop=sp)
                        nc.tensor.matmul(v_ps, wv_sb[:, k, :], rhs, start=st, stop=sp)
                    q_tmp = kv_stage.tile([128, 512], BF16, tag="qtmp")
                    nc.vector.tensor_copy(out=q_tmp, in_=q_ps)
                    k_tmp = kv_stage.tile([128, 512], BF16, tag="ktmp")
                    nc.vector.tensor_copy(out=k_tmp, in_=k_ps)
                    for h in range(HPC):
                        nc.sync.dma_start(
                            out=qp[h][ts(h, DH), ts(ch, 512)],
                            in_=q_tmp[ts(h, DH), :],
                        )
                        nc.sync.dma_start(
                            out=kp[h][ts(h, DH), ts(ch, 512)],
                            in_=k_tmp[ts(h, DH), :],
                        )
                    cp0(out=kT[:, ts(ch, 512)], in_=k_ps)
                    cp0(out=vT[:, ts(ch, 512)], in_=v_ps)

                vaug = []
                for h in range(HPC):
                    va = vaug_pool.tile(
                        [128, TBLK, 128], BF16, tag="vaug", name=f"va{b}_{h}"
                    )
                    nc.gpsimd.memset(va, 0.0)
                    nc.vector.tensor_copy(
                        out=va[:, :, DH : DH + 1],
                        in_=vinit_col.rearrange("p (n one) -> p n one", one=1),
                    )
                    vaug.append(va)
                for blk in range(TBLK):
                    vt_ps = trps.tile([128, 128], F32R, tag="trps")
                    nc.tensor.transpose(vt_ps, vT[:, ts(blk, 128)], ident)
                    vs_sb = kv_stage.tile([128, 128], F32R, tag="kvst")
                    cp0(out=vs_sb, in_=vt_ps)
                    nc.sync.dma_start(out=vo[b, ts(blk, 128), :], in_=vs_sb)
                    for h in range(HPC):
                        cp0(out=vaug[h][:, blk, 0:DH], in_=vt_ps[:, ts(h, DH)])
                return qp, kp, vaug

            def emit_proj_rs(b, outT, ch):
                for tt in range(4 * ch, 4 * ch + 4):
                    for dc in range(2):
                        p_ps = sc_ps.tile([128, 512], F32, tag="scps")
                        nc.tensor.matmul(
                            p_ps, outT[:, ts(tt, 128)],
                            wp_sb[:, ts(dc, 512)], start=True, stop=True,
                        )
                        p_sb = pr_stage.tile([128, 512], BF16, tag="prst")
                        nc.vector.tensor_copy(out=p_sb, in_=p_ps)
                        nc.sync.dma_start(
                            out=cc_in[b][ts(tt, 128), ts(dc, 512)], in_=p_sb
                        )
                if ch % 2 == 1:
                    hf = ch // 2
                    nc.gpsimd.collective_compute(
                        "ReduceScatter",
                        ALU.add,
                        ins=[cc_in[b][ts(hf, 1024), :]],
                        outs=[cc_out[b][hf]],
                        replica_groups=[list(range(NC))],
                    )

            def emit_attn_chunk(b, qp, kp, vaug, outT, ch):
                for h in range(HPC):
                    qh = qp[h]
                    kh = kp[h]
                    ntk = 4 * (ch + 1)
                    o_ps = pv_ps.tile([128, 512], F32, tag="pvps")
                    for tk in range(ntk):
                        s_ps = sc_ps.tile([128, 512], F32, tag="scps")
                        nc.tensor.matmul(
                            s_ps, kh[:, ts(tk, 128)], qh[:, ts(ch, 512)],
                            start=True, stop=True,
                        )
                        e_sb = exp_pool.tile([128, 512], BF16, tag="exp")
                        nc.scalar.activation(e_sb, s_ps, AF.Exp, scale=float(SCALE))
                        r = tk - 4 * ch
                        if r >= 0:
                            nc.vector.tensor_mul(e_sb, e_sb, mask_b[:, r, :])
                        nc.tensor.matmul(
                            o_ps, vaug[h][:, tk, :], e_sb,
                            start=(tk == 0), stop=(tk == ntk - 1),
                        )
                    l_sb = small.tile([1, 512], F32, tag="sc")
                    nc.vector.tensor_copy(out=l_sb, in_=o_ps[DH : DH + 1, :])
                    lb = rb_pool.tile([DH, 512], F32, tag="rb")
                    nc.gpsimd.partition_broadcast(lb, l_sb)
                    nc.vector.reciprocal_approx_fast(out=lb, in_=lb)
                    nc.vector.tensor_mul(
                        outT[ts(h, DH), ts(ch, 512)], o_ps[0:DH, :], lb
                    )

            # FF weight pools created early            # FF weight pools created early (right side) so their DMAs
            # prefetch during the attention era
            wg_pool = ctx.enter_context(tc.tile_pool(name="wg", bufs=2, side="right"))
            wu_pool = ctx.enter_context(tc.tile_pool(name="wu", bufs=2, side="right"))
            wd_pool = ctx.enter_context(tc.tile_pool(name="wd", bufs=3, side="right"))
            p0 = ExitStack()
            ss_ps = p0.enter_context(
                tc.tile_pool(name="ssps", bufs=2, space="PSUM", side="right")
            )
            scratch = p0.enter_context(
                tc.tile_pool(name="scratch", bufs=2, side="right")
            )
            # 4 causal mask variants: mask[p,j] = 1 if j - p - 128*r >= 0
            nc.vector.memset(mask_b, 1.0)
            nc.gpsimd.affine_select(
                out=mask_b, in_=mask_b,
                pattern=[[-128, 4], [1, 512]], compare_op=ALU.is_ge,
                fill=0.0, base=0, channel_multiplier=-1,
            )
            ht0 = emit_phase01(0, ss_ps, scratch)
            qkv_ps = actx.enter_context(tc.tile_pool(name="qkvps", bufs=3, space="PSUM"))
            qp0, kp0, vaug0 = emit_qkv(0, ht0)
            ht1 = emit_phase01(1, ss_ps, scratch)  # overlaps attention(b0)
            p0.close()
            sc_ps = actx.enter_context(tc.tile_pool(name="scps", bufs=2, space="PSUM"))
            pv_ps = actx.enter_context(tc.tile_pool(name="pvps", bufs=1, space="PSUM"))
            qp1, kp1, vaug1 = emit_qkv(1, ht1)
            outT0 = out2_pool.tile([128, T], BF16, tag="outT", name="outT0")
            outT1 = out2_pool.tile([128, T], BF16, tag="outT", name="outT1")
            for ch in range(TCH):
                emit_attn_chunk(0, qp0, kp0, vaug0, outT0, ch)
                emit_attn_chunk(1, qp1, kp1, vaug1, outT1, ch)
                # proj delayed by one chunk to hide the normalize latency
                if ch > 0:
                    emit_proj_rs(0, outT0, ch - 1)
                    emit_proj_rs(1, outT1, ch - 1)
            emit_proj_rs(0, outT0, TCH - 1)
            emit_proj_rs(1, outT1, TCH - 1)

        # =====================  phase 5 + FFN era  ==========================
        xa_pool = ctx.enter_context(tc.tile_pool(name="xa", bufs=4))
        h2t_pool = ctx.enter_context(tc.tile_pool(name="h2t", bufs=2))
        h2T = [
            h2t_pool.tile([128, 4 * SH], BF16, tag="h2t", name=f"h2T{g}")
            for g in range(2)
        ]

        def h2t_sl(k, tt):
            off = (k % 4) * SH + tt * 128
            return h2T[k // 4][:, off : off + 128]

        def h2T_full(k):
            off = (k % 4) * SH
            return h2T[k // 4][:, off : off + SH]

        # ---- deferred k-cache transposes + writes ----
        with ExitStack() as p8:
            kst = p8.enter_context(tc.tile_pool(name="kst", bufs=3))
            for b in range(B):
                for blk in range(TBLK):
                    kt_ps = trps.tile([128, 128], F32R, tag="trps")
                    nc.tensor.transpose(kt_ps, kT_cache[b][:, ts(blk, 128)], ident)
                    ks_sb = kst.tile([128, 128], F32R, tag="kst")
                    nc.vector.tensor_copy(out=ks_sb, in_=kt_ps)
                    nc.sync.dma_start(out=ko[b, ts(blk, 128), :], in_=ks_sb)


        xa = []

        def emit_phase5(tt, n2_pool, sm5, h2_pool):
            bsel, hf = tt // 2, tt % 2
            xs_t = n2_pool.tile([128, D], F32, tag="xst")
            nc.sync.dma_start(out=xs_t, in_=xs[ts(tt, 128), :])
            cc_t = n2_pool.tile([128, D], BF16, tag="cct")
            nc.sync.dma_start(out=cc_t, in_=cc_out[bsel][hf])
            xat = xa_pool.tile([128, D], F32, tag="xa", name=f"xa{tt}")
            nc.vector.tensor_add(xat, xs_t, cc_t)
            xa.append(xat)

            sq2 = n2_pool.tile([128, D], F32, tag="sq2")
            ms = sm5.tile([128, 1], F32, tag="s5")
            nc.scalar.activation(sq2, xat, AF.Square, accum_out=ms)
            srt = sm5.tile([128, 1], F32, tag="s5")
            nc.scalar.activation(srt, ms, AF.Sqrt, bias=eps128, scale=1.0 / D)
            rs2 = sm5.tile([128, 1], F32, tag="s5")
            nc.vector.reciprocal_approx_fast(out=rs2, in_=srt)
            h2 = h2_pool.tile([128, D], F32R, tag="h2")
            nc.vector.scalar_tensor_tensor(
                out=h2, in0=xat, scalar=rs2, in1=w2_sb,
                op0=ALU.mult, op1=ALU.mult,
            )
            for k in range(KT):
                t_ps = trps.tile([128, 128], F32R, tag="trps")
                nc.tensor.transpose(t_ps, h2[:, ts(k, 128)], ident)
                nc.vector.tensor_copy(out=h2t_sl(k, tt), in_=t_ps)

        # ---- FFN gate/up -> guT (bf16) ----
        gut_pool = ctx.enter_context(tc.tile_pool(name="gut", bufs=FFC))
        guT = [
            gut_pool.tile([128, 4 * SH], BF16, tag="gut", name=f"guT{g}")
            for g in range(FFC)
        ]

        def gut_sl(f, tt):
            off = (f % 4) * SH + tt * 128
            return guT[f // 4][:, off : off + 128]

        with ExitStack() as p6:
            n2_pool = p6.enter_context(tc.tile_pool(name="n2", bufs=2))
            h2_pool = p6.enter_context(tc.tile_pool(name="h2", bufs=2))
            sm5 = p6.enter_context(tc.tile_pool(name="sm5", bufs=2))
            gsil_pool = p6.enter_context(tc.tile_pool(name="gsil", bufs=2))
            gu_pool = p6.enter_context(tc.tile_pool(name="gu", bufs=2))
            ff_ps = p6.enter_context(tc.tile_pool(name="ffps", bufs=3, space="PSUM"))
            wg_r = wg.rearrange("(k p) f -> p k f", p=128)
            wu_r = wu.rearrange("(k p) f -> p k f", p=128)

            for tt in range(4):
                emit_phase5(tt, n2_pool, sm5, h2_pool)

            for fc in range(FFC):
                wgt = wg_pool.tile([128, KT, 512], BF16, tag="wg", name=f"wg{fc}")
                nc.sync.dma_start(out=wgt, in_=wg_r[:, :, ts(fc, 512)])
                wut = wu_pool.tile([128, KT, 512], BF16, tag="wu", name=f"wu{fc}")
                nc.sync.dma_start(out=wut, in_=wu_r[:, :, ts(fc, 512)])
                for fj in range(4):
                    f = fc * 4 + fj
                    g_ps = ff_ps.tile([128, 512], F32, tag="ffps")
                    for k in range(KT):
                        nc.tensor.matmul(
                            g_ps, wgt[:, k, ts(fj, 128)], h2T_full(k),
                            start=(k == 0), stop=(k == KT - 1),
                        )
                    g_si = gsil_pool.tile([128, 512], F32R, tag="gsil")
                    nc.scalar.activation(g_si, g_ps, AF.Silu)
                    u_ps = ff_ps.tile([128, 512], F32, tag="ffps")
                    for k in range(KT):
                        nc.tensor.matmul(
                            u_ps, wut[:, k, ts(fj, 128)], h2T_full(k),
                            start=(k == 0), stop=(k == KT - 1),
                        )
                    gu_out = guT[f // 4][:, (f % 4) * SH : (f % 4 + 1) * SH]
                    nc.vector.tensor_mul(gu_out, u_ps, g_si)

        # ---- FFN down + residual -> xo ----
        with ExitStack() as p7:
            d_ps_pool = p7.enter_context(tc.tile_pool(name="dps", bufs=4, space="PSUM"))
            out_pool = p7.enter_context(tc.tile_pool(name="osb", bufs=4))
            for dc in range(2):
                d_ps = [
                    d_ps_pool.tile([128, 512], F32, tag="dps", name=f"dps{dc}_{i}")
                    for i in range(4)
                ]
                for f in range(FFT):
                    wdt = wd_pool.tile([128, 512], BF16, tag="wd")
                    nc.sync.dma_start(out=wdt, in_=wd[ts(f, 128), ts(dc, 512)])
                    for tt in range(4):
                        nc.tensor.matmul(
                            d_ps[tt], gut_sl(f, tt), wdt,
                            start=(f == 0), stop=(f == FFT - 1),
                        )
                for tt in range(4):
                    o_sb = out_pool.tile([128, 512], F32, tag="osb")
                    nc.vector.tensor_add(o_sb, d_ps[tt], xa[tt][:, ts(dc, 512)])
                    nc.sync.dma_start(out=xo[ts(tt, 128), ts(dc, 512)], in_=o_sb)

def _build():
    if "nc" in _BUILT:
        return _BUILT["nc"]
    nc = bacc.Bacc("TRN2", target_bir_lowering=False, debug=False, num_devices=NC)
    io = {}
    io["xt"] = nc.dram_tensor("xt", [B, D, T], BF16, kind="ExternalInput").ap()
    io["xs"] = nc.dram_tensor("xs", [SH, D], F32, kind="ExternalInput").ap()
    io["wq_t"] = nc.dram_tensor("wq_t", [D, HD], BF16, kind="ExternalInput").ap()
    io["wk_t"] = nc.dram_tensor("wk_t", [D, HD], BF16, kind="ExternalInput").ap()
    io["wv_t"] = nc.dram_tensor("wv_t", [D, HD], BF16, kind="ExternalInput").ap()
    io["wp_t"] = nc.dram_tensor("wp_t", [HD, D], BF16, kind="ExternalInput").ap()
    io["w1r"] = nc.dram_tensor("w1r", [128, KT], F32, kind="ExternalInput").ap()
    io["w2"] = nc.dram_tensor("w2", [D], F32, kind="ExternalInput").ap()
    io["wg_t"] = nc.dram_tensor("wg_t", [D, DFF], BF16, kind="ExternalInput").ap()
    io["wu_t"] = nc.dram_tensor("wu_t", [D, DFF], BF16, kind="ExternalInput").ap()
    io["wd_t"] = nc.dram_tensor("wd_t", [DFF, D], BF16, kind="ExternalInput").ap()
    io["xo"] = nc.dram_tensor("xo", [SH, D], F32, kind="ExternalOutput").ap()
    io["ko"] = nc.dram_tensor("ko", [B, T, HD], F32R, kind="ExternalOutput").ap()
    io["vo"] = nc.dram_tensor("vo", [B, T, HD], F32R, kind="ExternalOutput").ap()
    io["cc_in0"] = nc.dram_tensor("cc_in0", [T, D], BF16, kind="Internal").ap()
    io["cc_in1"] = nc.dram_tensor("cc_in1", [T, D], BF16, kind="Internal").ap()
    for b in range(B):
        for hf in range(2):
            io[f"cc_out{b}{hf}"] = nc.dram_tensor(
                f"cc_out{b}{hf}", [128, D], BF16, kind="Internal"
            ).ap()

    _emit(nc, io)
    nc.compile()
    _BUILT["nc"] = nc
    return nc


def _rows(tt, c):
    """Global rows of core c's tt-th 128-token block (one 128-row piece).

    (b, hf) = (tt//2, tt%2); the ReduceScatter over rows [1024*hf, 1024*(hf+1))
    of batch b gives core c rows 128*c of that range.
    """
    b, hf = tt // 2, tt % 2
    return [b * T + hf * 1024 + 128 * c]


def kernel(x, w_norm1, w_qkv, w_proj, w_norm2, w_gate, w_up, w_down, step=0):
    x = np.asarray(x, dtype=np.float32)
    w_norm1 = np.asarray(w_norm1, dtype=np.float32)
    w_qkv = np.asarray(w_qkv, dtype=np.float32)
    w_proj = np.asarray(w_proj, dtype=np.float32)
    w_norm2 = np.asarray(w_norm2, dtype=np.float32)
    w_gate = np.asarray(w_gate, dtype=np.float32)
    w_up = np.asarray(w_up, dtype=np.float32)
    w_down = np.asarray(w_down, dtype=np.float32)

    nc = _build()

    import ml_dtypes as _mld
    xt = np.ascontiguousarray(x.transpose(0, 2, 1)).astype(_mld.bfloat16)  # [B, D, T]
    xf = x.reshape(B * T, D)
    import ml_dtypes
    bf16 = ml_dtypes.bfloat16
    w1r = np.ascontiguousarray(w_norm1.reshape(KT, 128).T)    # [128, KT]
    wg_t = np.ascontiguousarray(w_gate.T).astype(bf16)        # [D, DFF]
    wu_t = np.ascontiguousarray(w_up.T).astype(bf16)          # [D, DFF]
    wd_t = np.ascontiguousarray(w_down.T).astype(bf16)        # [DFF, D]

    in_maps = []
    for c in range(NC):
        xs_c = np.concatenate(
            [xf[g : g + 128] for tt in range(4) for g in _rows(tt, c)], axis=0
        )
        in_maps.append({
            "xt": xt,
            "xs": np.ascontiguousarray(xs_c),
            "wq_t": np.ascontiguousarray(w_qkv[HD * c : HD * (c + 1)].T).astype(bf16),
            "wk_t": np.ascontiguousarray(w_qkv[D + HD * c : D + HD * (c + 1)].T).astype(bf16),
            "wv_t": np.ascontiguousarray(
                w_qkv[2 * D + HD * c : 2 * D + HD * (c + 1)].T
            ).astype(bf16),
            "wp_t": np.ascontiguousarray(w_proj[:, HD * c : HD * (c + 1)].T).astype(bf16),
            "w1r": w1r,
            "w2": w_norm2,
            "wg_t": wg_t,
            "wu_t": wu_t,
            "wd_t": wd_t,
        })

    global _last_in_maps
    _last_in_maps = in_maps
    res = run_bass_kernel_spmd(nc, in_maps, core_ids=list(range(NC)))

    x_out = np.empty((B * T, D), dtype=np.float32)
    k_cache = np.empty((B, H, T, DH), dtype=np.float32)
    v_cache = np.empty((B, H, T, DH), dtype=np.float32)
    for c in range(NC):
        out_c = res.results[c]
        for tt in range(4):
            g = _rows(tt, c)[0]
            x_out[g : g + 128] = out_c["xo"][128 * tt : 128 * (tt + 1)]
        kc = out_c["ko"].reshape(B, T, HPC, DH).transpose(0, 2, 1, 3)
        vc = out_c["vo"].reshape(B, T, HPC, DH).transpose(0, 2, 1, 3)
        k_cache[:, HPC * c : HPC * (c + 1)] = kc
        v_cache[:, HPC * c : HPC * (c + 1)] = vc

    return x_out.reshape(B, T, D), k_cache, v_cache


# revision 37
# speedup vs baseline: 1.0906x; 1.0906x over previous
"""Trainium2 Bass kernel for a dense transformer block (prefill), SPMD on 8 NeuronCores.

Sharding strategy:
  - Attention: tensor-parallel over heads (16 heads -> 2 per core), both batches
    on every core. Each core computes RMSNorm(x) -> qkv for its heads -> causal
    attention -> output-projection partial (row-parallel w_proj slice).
  - Proj partials are ReduceScatter-summed over the 8 cores (one RS per batch,
    scattered over the token dim), so core c ends up owning 256 tokens of each
    batch (512 total).
  - FFN: token-parallel. Each core runs residual + RMSNorm2 + gate/up/down over
    its 512 tokens with the full FF weights (streamed from HBM).
  - Host side only does layout work: transposes/slices inputs, concatenates
    output shards.

Matmuls run in bf16 (k/v cache values and all accumulations stay fp32;
the two ReduceScatters per batch move bf16 partials).
"""

import numpy as np
from contextlib import ExitStack

import concourse.bacc as bacc
import concourse.bass as bass
import concourse.tile as tile
import concourse.mybir as mybir
from concourse.bass_utils import run_bass_kernel_spmd
from concourse.masks import make_identity

F32R = mybir.dt.float32r
F32 = mybir.dt.float32
BF16 = mybir.dt.bfloat16
AF = mybir.ActivationFunctionType
ALU = mybir.AluOpType
ts = bass.ts

B, T, D = 2, 2048, 1024
H, DH = 16, 64
NC = 8
HPC = H // NC            # heads per core = 2
HD = HPC * DH            # 128 head-dims per core
KT = D // 128            # 8 k-tiles over D
TCH = T // 512           # 4 query chunks per batch
TBLK = T // 128          # 16 token tiles per batch
SHB = T // NC            # 256 tokens per (core, batch) after RS
SH = B * SHB             # 512 tokens per core shard
DFF = 4096
FFC = DFF // 512         # 8 ff chunks
FFT = DFF // 128         # 32 ff tiles
EPS = 1e-6
SCALE = 1.0 / np.sqrt(DH)

_BUILT = {}


def _emit(nc, io):
    xt, xs, wq, wk, wv, wp, w1, w2, wg, wu, wd = (
        io["xt"], io["xs"], io["wq_t"], io["wk_t"], io["wv_t"], io["wp_t"],
        io["w1r"], io["w2"], io["wg_t"], io["wu_t"], io["wd_t"],
    )
    xo, ko, vo = io["xo"], io["ko"], io["vo"]
    cc_in = [io["cc_in0"], io["cc_in1"]]
    cc_out = [
        [io[f"cc_out{b}{hf}"] for hf in range(2)] for b in range(B)
    ]

    with tile.TileContext(nc) as tc, ExitStack() as ctx:
        ctx.enter_context(
            nc.allow_low_precision(reason="bf16/fp32r intermediates for PE matmuls")
        )
        const = ctx.enter_context(tc.tile_pool(name="const", bufs=1))
        trps = ctx.enter_context(tc.tile_pool(name="trps", bufs=2, space="PSUM"))
        cache_pool = ctx.enter_context(tc.tile_pool(name="cache", bufs=2))

        ident_f = const.tile([128, 128], F32)
        make_identity(nc, ident_f)
        ident = const.tile([128, 128], F32R)
        nc.vector.tensor_copy(out=ident, in_=ident_f)
        eps128 = const.tile([128, 1], F32)
        nc.vector.memset(eps128, EPS)
        w2_sb = const.tile([128, D], F32)
        nc.sync.dma_start(
            out=w2_sb,
            in_=bass.AP(tensor=w2.tensor, offset=0, ap=[[0, 128], [1, D]]),
        )

        kT_cache = []

        # =====================  attention era  ==============================
        with ExitStack() as actx:
            catt = actx.enter_context(tc.tile_pool(name="catt", bufs=1))
            ones_col = catt.tile([128, 128], BF16)
            nc.vector.memset(ones_col, 1.0)
            vinit_col = catt.tile([128, TBLK], F32)
            nc.vector.memset(vinit_col, 1.0)
            mask_b = catt.tile([128, 4, 512], BF16)
            eps1 = catt.tile([1, 1], F32)
            nc.vector.memset(eps1, EPS)
            w1_sb = catt.tile([128, KT], F32)
            nc.sync.dma_start(out=w1_sb, in_=w1)
            wq_sb = catt.tile([128, KT, HD], BF16)
            nc.sync.dma_start(out=wq_sb, in_=wq.rearrange("(k p) m -> p k m", p=128))
            wk_sb = catt.tile([128, KT, HD], BF16)
            nc.sync.dma_start(out=wk_sb, in_=wk.rearrange("(k p) m -> p k m", p=128))
            wv_sb = catt.tile([128, KT, HD], BF16)
            nc.sync.dma_start(out=wv_sb, in_=wv.rearrange("(k p) m -> p k m", p=128))
            wp_sb = catt.tile([128, D], BF16)
            nc.sync.dma_start(out=wp_sb, in_=wp)

            small = actx.enter_context(tc.tile_pool(name="small", bufs=3))
            rsb_pool = actx.enter_context(tc.tile_pool(name="rsb", bufs=2))
            ht_pool = actx.enter_context(tc.tile_pool(name="ht", bufs=KT))

            # ---- phase 0: RMSNorm1 stats (both batches) ----
            rs_bs = []
            xk_b0 = []
            def emit_phase01(b, ss_ps, scratch):
                ss = [
                    ss_ps.tile([128, 512], F32, tag="ssq", name=f"ss{b}_{i}")
                    for i in range(TCH)
                ]
                rsb = rsb_pool.tile([128, T], F32, tag="rsb", name=f"rsb{b}")
                rs_bs.append(rsb)
                xk = [
                    ht_pool.tile([128, T], BF16, tag="ht", name=f"ht{b}_{k}")
                    for k in range(KT)
                ]
                for ch in range(TCH):
                    for k in range(KT):
                        nc.sync.dma_start(
                            out=xk[k][:, ts(ch, 512)],
                            in_=xt[b, ts(k, 128), ts(ch, 512)],
                        )
                        sq = scratch.tile([128, 512], BF16, tag="sqs")
                        nc.vector.tensor_mul(sq, xk[k][:, ts(ch, 512)],
                                             xk[k][:, ts(ch, 512)])
                        nc.tensor.matmul(
                            ss[ch], ones_col, sq,
                            start=(k == 0), stop=(k == KT - 1),
                        )
                    srt = small.tile([1, 512], F32, tag="sc")
                    nc.scalar.activation(srt, ss[ch][0:1, :], AF.Sqrt, bias=eps1,
                                         scale=1.0 / D)
                    nc.gpsimd.partition_broadcast(rsb[:, ts(ch, 512)], srt)
                    nc.vector.reciprocal_approx_fast(
                        out=rsb[:, ts(ch, 512)], in_=rsb[:, ts(ch, 512)]
                    )
                    for k in range(KT):
                        nc.vector.scalar_tensor_tensor(
                            out=xk[k][:, ts(ch, 512)],
                            in0=xk[k][:, ts(ch, 512)],
                            scalar=w1_sb[:, k : k + 1],
                            in1=rsb[:, ts(ch, 512)],
                            op0=ALU.mult, op1=ALU.mult,
                        )
                return xk

            qkv_sb = actx.enter_context(tc.tile_pool(name="qkvsb", bufs=2))
            vaug_pool = actx.enter_context(tc.tile_pool(name="vaug", bufs=2))
            out2_pool = actx.enter_context(tc.tile_pool(name="out2", bufs=2))
            exp_pool = actx.enter_context(tc.tile_pool(name="exp", bufs=6))
            rb_pool = actx.enter_context(tc.tile_pool(name="rb", bufs=2))
            kv_stage = actx.enter_context(tc.tile_pool(name="kvst", bufs=2))
            pr_stage = actx.enter_context(tc.tile_pool(name="prst", bufs=3))

            def emit_qkv(b, ht):
                qp = [
                    qkv_sb.tile([128, T], BF16, tag=f"qp{h}", name=f"qp{b}_{h}", bufs=1)
                    for h in range(HPC)
                ]
                kp = [
                    qkv_sb.tile([128, T], BF16, tag=f"kp{h}", name=f"kp{b}_{h}", bufs=1)
                    for h in range(HPC)
                ]
                for h in range(HPC):
                    nc.gpsimd.memset(qp[h][ts(1 - h, DH), :], 0.0)
                    nc.gpsimd.memset(kp[h][ts(1 - h, DH), :], 0.0)
                kT = cache_pool.tile([128, T], F32R, tag="kT", name=f"kT{b}")
                kT_cache.append(kT)
                vT = qkv_sb.tile([128, T], F32R, tag="vT", name=f"vT{b}")
                cp0 = nc.scalar.copy if b == 0 else (
                    lambda out, in_: nc.vector.tensor_copy(out=out, in_=in_)
                )
                for ch in range(TCH):
                    q_ps = qkv_ps.tile([128, 512], F32, tag="qkvps")
                    k_ps = qkv_ps.tile([128, 512], F32, tag="qkvps")
                    v_ps = qkv_ps.tile([128, 512], F32, tag="qkvps")
                    for k in range(KT):
                        st = (k == 0)
                        sp = (k == KT - 1)
                        rhs = ht[k][:, ts(ch, 512)]
                        nc.tensor.matmul(q_ps, wq_sb[:, k, :], rhs, start=st, stop=sp)
                        nc.tensor.matmul(k_ps, wk_sb[:, k, :], rhs, start=st, stop=sp)
                        nc.tensor.matmul(v_ps, wv_sb[:, k, :], rhs, start=st, stop=sp)
                    q_tmp = kv_stage.tile([128, 512], BF16, tag="qtmp")
                    nc.vector.tensor_copy(out=q_tmp, in_=q_ps)
                    k_tmp = kv_stage.tile([128, 512], BF16, tag="ktmp")
                    nc.vector.tensor_copy(out=k_tmp, in_=k_ps)
                    for h in range(HPC):
                        nc.sync.dma_start(
                            out=qp[h][ts(h, DH), ts(ch, 512)],
                            in_=q_tmp[ts(h, DH), :],
                        )
                        nc.sync.dma_start(
                            out=kp[h][ts(h, DH), ts(ch, 512)],
                            in_=k_tmp[ts(h, DH), :],
                        )
                    cp0(out=kT[:, ts(ch, 512)], in_=k_ps)
                    cp0(out=vT[:, ts(ch, 512)], in_=v_ps)

                vaug = []
                for h in range(HPC):
                    va = vaug_pool.tile(
                        [128, TBLK, 128], BF16, tag="vaug", name=f"va{b}_{h}"
                    )
                    nc.gpsimd.memset(va, 0.0)
                    nc.vector.tensor_copy(
                        out=va[:, :, DH : DH + 1],
                        in_=vinit_col.rearrange("p (n one) -> p n one", one=1),
                    )
                    vaug.append(va)
                for blk in range(TBLK):
                    vt_ps = trps.tile([128, 128], F32R, tag="trps")
                    nc.tensor.transpose(vt_ps, vT[:, ts(blk, 128)], ident)
                    vs_sb = kv_stage.tile([128, 128], F32R, tag="kvst")
                    cp0(out=vs_sb, in_=vt_ps)
                    nc.sync.dma_start(out=vo[b, ts(blk, 128), :], in_=vs_sb)
                    for h in range(HPC):
                        cp0(out=vaug[h][:, blk, 0:DH], in_=vt_ps[:, ts(h, DH)])
                return qp, kp, vaug

            def emit_proj_rs(b, outT, ch):
                for tt in range(4 * ch, 4 * ch + 4):
                    for dc in range(2):
                        p_ps = sc_ps.tile([128, 512], F32, tag="scps")
                        nc.tensor.matmul(
                            p_ps, outT[:, ts(tt, 128)],
                            wp_sb[:, ts(dc, 512)], start=True, stop=True,
                        )
                        p_sb = pr_stage.tile([128, 512], BF16, tag="prst")
                        nc.vector.tensor_copy(out=p_sb, in_=p_ps)
                        nc.sync.dma_start(
                            out=cc_in[b][ts(tt, 128), ts(dc, 512)], in_=p_sb
                        )
                if ch % 2 == 1:
                    hf = ch // 2
                    nc.gpsimd.collective_compute(
                        "ReduceScatter",
                        ALU.add,
                        ins=[cc_in[b][ts(hf, 1024), :]],
                        outs=[cc_out[b][hf]],
                        replica_groups=[list(range(NC))],
                    )

            def emit_attn(b, qp, kp, vaug):
                outT = out2_pool.tile([128, T], BF16, tag="outT", name=f"outT{b}")
                for ch in range(TCH):
                    for h in range(HPC):
                        qh = qp[h]
                        kh = kp[h]
                        ntk = 4 * (ch + 1)
                        o_ps = pv_ps.tile([128, 512], F32, tag="pvps")
                        for tk in range(ntk):
                            s_ps = sc_ps.tile([128, 512], F32, tag="scps")
                            nc.tensor.matmul(
                                s_ps, kh[:, ts(tk, 128)], qh[:, ts(ch, 512)],
                                start=True, stop=True,
                            )
                            e_sb = exp_pool.tile([128, 512], BF16, tag="exp")
                            nc.scalar.activation(e_sb, s_ps, AF.Exp, scale=float(SCALE))
                            r = tk - 4 * ch
                            if r >= 0:
                                nc.vector.tensor_mul(e_sb, e_sb, mask_b[:, r, :])
                            nc.tensor.matmul(
                                o_ps, vaug[h][:, tk, :], e_sb,
                                start=(tk == 0), stop=(tk == ntk - 1),
                            )
                        l_sb = small.tile([1, 512], F32, tag="sc")
                        nc.vector.tensor_copy(out=l_sb, in_=o_ps[DH : DH + 1, :])
                        lb = rb_pool.tile([DH, 512], F32, tag="rb")
                        nc.gpsimd.partition_broadcast(lb, l_sb)
                        nc.vector.reciprocal_approx_fast(out=lb, in_=lb)
                        nc.vector.tensor_mul(
                            outT[ts(h, DH), ts(ch, 512)], o_ps[0:DH, :], lb
                        )
                    # proj delayed by one chunk to hide the normalize latency
                    if ch > 0:
                        emit_proj_rs(b, outT, ch - 1)
                emit_proj_rs(b, outT, TCH - 1)

            # FF weight pools created early (right side) so their DMAs
            # prefetch during the attention era
            wg_pool = ctx.enter_context(tc.tile_pool(name="wg", bufs=2, side="right"))
            wu_pool = ctx.enter_context(tc.tile_pool(name="wu", bufs=2, side="right"))
            wd_pool = ctx.enter_context(tc.tile_pool(name="wd", bufs=4, side="right"))
            p0 = ExitStack()
            ss_ps = p0.enter_context(
                tc.tile_pool(name="ssps", bufs=2, space="PSUM", side="right")
            )
            scratch = p0.enter_context(
                tc.tile_pool(name="scratch", bufs=2, side="right")
            )
            # 4 causal mask variants: mask[p,j] = 1 if j - p - 128*r >= 0
            nc.vector.memset(mask_b, 1.0)
            nc.gpsimd.affine_select(
                out=mask_b, in_=mask_b,
                pattern=[[-128, 4], [1, 512]], compare_op=ALU.is_ge,
                fill=0.0, base=0, channel_multiplier=-1,
            )
            ht0 = emit_phase01(0, ss_ps, scratch)
            qkv_ps = actx.enter_context(tc.tile_pool(name="qkvps", bufs=3, space="PSUM"))
            qp0, kp0, vaug0 = emit_qkv(0, ht0)
            ht1 = emit_phase01(1, ss_ps, scratch)  # overlaps attention(b0)
            p0.close()
            sc_ps = actx.enter_context(tc.tile_pool(name="scps", bufs=2, space="PSUM"))
            pv_ps = actx.enter_context(tc.tile_pool(name="pvps", bufs=1, space="PSUM"))
            emit_attn(0, qp0, kp0, vaug0)
            qp1, kp1, vaug1 = emit_qkv(1, ht1)
            emit_attn(1, qp1, kp1, vaug1)

        # =====================  phase 5 + FFN era  ==========================
        xa_pool = ctx.enter_context(tc.tile_pool(name="xa", bufs=4))
        h2t_pool = ctx.enter_context(tc.tile_pool(name="h2t", bufs=2))
        h2T = [
            h2t_pool.tile([128, 4 * SH], BF16, tag="h2t", name=f"h2T{g}")
            for g in range(2)
        ]

        def h2t_sl(k, tt):
            off = (k % 4) * SH + tt * 128
            return h2T[k // 4][:, off : off + 128]

        def h2T_full(k):
            off = (k % 4) * SH
            return h2T[k // 4][:, off : off + SH]

        # ---- deferred k-cache transposes + writes ----
        with ExitStack() as p8:
            kst = p8.enter_context(tc.tile_pool(name="kst", bufs=3))
            for b in range(B):
                for blk in range(TBLK):
                    kt_ps = trps.tile([128, 128], F32R, tag="trps")
                    nc.tensor.transpose(kt_ps, kT_cache[b][:, ts(blk, 128)], ident)
                    ks_sb = kst.tile([128, 128], F32R, tag="kst")
                    nc.vector.tensor_copy(out=ks_sb, in_=kt_ps)
                    nc.sync.dma_start(out=ko[b, ts(blk, 128), :], in_=ks_sb)


        xa = []

        def emit_phase5(tt, n2_pool, sm5, h2_pool):
            bsel, hf = tt // 2, tt % 2
            xs_t = n2_pool.tile([128, D], F32, tag="xst")
            nc.sync.dma_start(out=xs_t, in_=xs[ts(tt, 128), :])
            cc_t = n2_pool.tile([128, D], BF16, tag="cct")
            nc.sync.dma_start(out=cc_t, in_=cc_out[bsel][hf])
            xat = xa_pool.tile([128, D], F32, tag="xa", name=f"xa{tt}")
            nc.vector.tensor_add(xat, xs_t, cc_t)
            xa.append(xat)

            sq2 = n2_pool.tile([128, D], F32, tag="sq2")
            ms = sm5.tile([128, 1], F32, tag="s5")
            nc.scalar.activation(sq2, xat, AF.Square, accum_out=ms)
            srt = sm5.tile([128, 1], F32, tag="s5")
            nc.scalar.activation(srt, ms, AF.Sqrt, bias=eps128, scale=1.0 / D)
            rs2 = sm5.tile([128, 1], F32, tag="s5")
            nc.vector.reciprocal_approx_fast(out=rs2, in_=srt)
            h2 = h2_pool.tile([128, D], F32R, tag="h2")
            nc.vector.scalar_tensor_tensor(
                out=h2, in0=xat, scalar=rs2, in1=w2_sb,
                op0=ALU.mult, op1=ALU.mult,
            )
            for k in range(KT):
                t_ps = trps.tile([128, 128], F32R, tag="trps")
                nc.tensor.transpose(t_ps, h2[:, ts(k, 128)], ident)
                nc.vector.tensor_copy(out=h2t_sl(k, tt), in_=t_ps)

        # ---- FFN gate/up -> guT (bf16) ----
        gut_pool = ctx.enter_context(tc.tile_pool(name="gut", bufs=FFC))
        guT = [
            gut_pool.tile([128, 4 * SH], BF16, tag="gut", name=f"guT{g}")
            for g in range(FFC)
        ]

        def gut_sl(f, tt):
            off = (f % 4) * SH + tt * 128
            return guT[f // 4][:, off : off + 128]

        with ExitStack() as p6:
            n2_pool = p6.enter_context(tc.tile_pool(name="n2", bufs=2))
            h2_pool = p6.enter_context(tc.tile_pool(name="h2", bufs=2))
            sm5 = p6.enter_context(tc.tile_pool(name="sm5", bufs=2))
            gsil_pool = p6.enter_context(tc.tile_pool(name="gsil", bufs=2))
            gu_pool = p6.enter_context(tc.tile_pool(name="gu", bufs=2))
            ff_ps = p6.enter_context(tc.tile_pool(name="ffps", bufs=3, space="PSUM"))
            wg_r = wg.rearrange("(k p) f -> p k f", p=128)
            wu_r = wu.rearrange("(k p) f -> p k f", p=128)

            for tt in range(4):
                emit_phase5(tt, n2_pool, sm5, h2_pool)

            for fc in range(FFC):
                wgt = wg_pool.tile([128, KT, 512], BF16, tag="wg", name=f"wg{fc}")
                nc.sync.dma_start(out=wgt, in_=wg_r[:, :, ts(fc, 512)])
                wut = wu_pool.tile([128, KT, 512], BF16, tag="wu", name=f"wu{fc}")
                nc.sync.dma_start(out=wut, in_=wu_r[:, :, ts(fc, 512)])
                for fj in range(4):
                    f = fc * 4 + fj
                    g_ps = ff_ps.tile([128, 512], F32, tag="ffps")
                    for k in range(KT):
                        nc.tensor.matmul(
                            g_ps, wgt[:, k, ts(fj, 128)], h2T_full(k),
                            start=(k == 0), stop=(k == KT - 1),
                        )
                    g_si = gsil_pool.tile([128, 512], F32R, tag="gsil")
                    nc.scalar.activation(g_si, g_ps, AF.Silu)
                    u_ps = ff_ps.tile([128, 512], F32, tag="ffps")
                    for k in range(KT):
                        nc.tensor.matmul(
                            u_ps, wut[:, k, ts(fj, 128)], h2T_full(k),
                            start=(k == 0), stop=(k == KT - 1),
                        )
                    gu_out = guT[f // 4][:, (f % 4) * SH : (f % 4 + 1) * SH]
                    nc.vector.tensor_mul(gu_out, u_ps, g_si)

        # ---- FFN down + residual -> xo ----
        with ExitStack() as p7:
            d_ps_pool = p7.enter_context(tc.tile_pool(name="dps", bufs=4, space="PSUM"))
            out_pool = p7.enter_context(tc.tile_pool(name="osb", bufs=4))
            for dc in range(2):
                d_ps = [
                    d_ps_pool.tile([128, 512], F32, tag="dps", name=f"dps{dc}_{i}")
                    for i in range(4)
                ]
                for f in range(FFT):
                    wdt = wd_pool.tile([128, 512], BF16, tag="wd")
                    nc.sync.dma_start(out=wdt, in_=wd[ts(f, 128), ts(dc, 512)])
                    for tt in range(4):
                        nc.tensor.matmul(
                            d_ps[tt], gut_sl(f, tt), wdt,
                            start=(f == 0), stop=(f == FFT - 1),
                        )
                for tt in range(4):
                    o_sb = out_pool.tile([128, 512], F32, tag="osb")
                    nc.vector.tensor_add(o_sb, d_ps[tt], xa[tt][:, ts(dc, 512)])
                    nc.sync.dma_start(out=xo[ts(tt, 128), ts(dc, 512)], in_=o_sb)

def _build():
    if "nc" in _BUILT:
        return _BUILT["nc"]
    nc = bacc.Bacc("TRN2", target_bir_lowering=False, debug=False, num_devices=NC)
    io = {}
    io["xt"] = nc.dram_tensor("xt", [B, D, T], BF16, kind="ExternalInput").ap()
    io["xs"] = nc.dram_tensor("xs", [SH, D], F32, kind="ExternalInput").ap()
    io["wq_t"] = nc.dram_tensor("wq_t", [D, HD], BF16, kind="ExternalInput").ap()
    io["wk_t"] = nc.dram_tensor("wk_t", [D, HD], BF16, kind="ExternalInput").ap()
    io["wv_t"] = nc.dram_tensor("wv_t", [D, HD], BF16, kind="ExternalInput").ap()
    io["wp_t"] = nc.dram_tensor("wp_t", [HD, D], BF16, kind="ExternalInput").ap()
    io["w1r"] = nc.dram_tensor("w1r", [128, KT], F32, kind="ExternalInput").ap()
    io["w2"] = nc.dram_tensor("w2", [D], F32, kind="ExternalInput").ap()
    io["wg_t"] = nc.dram_tensor("wg_t", [D, DFF], BF16, kind="ExternalInput").ap()
    io["wu_t"] = nc.dram_tensor("wu_t", [D, DFF], BF16, kind="ExternalInput").ap()
    io["wd_t"] = nc.dram_tensor("wd_t", [DFF, D], BF16, kind="ExternalInput").ap()
    io["xo"] = nc.dram_tensor("xo", [SH, D], F32, kind="ExternalOutput").ap()
    io["ko"] = nc.dram_tensor("ko", [B, T, HD], F32R, kind="ExternalOutput").ap()
    io["vo"] = nc.dram_tensor("vo", [B, T, HD], F32R, kind="ExternalOutput").ap()
    io["cc_in0"] = nc.dram_tensor("cc_in0", [T, D], BF16, kind="Internal").ap()
    io["cc_in1"] = nc.dram_tensor("cc_in1", [T, D], BF16, kind="Internal").ap()
    for b in range(B):
        for hf in range(2):
            io[f"cc_out{b}{hf}"] = nc.dram_tensor(
                f"cc_out{b}{hf}", [128, D], BF16, kind="Internal"
            ).ap()

    _emit(nc, io)
    nc.compile()
    _BUILT["nc"] = nc
    return nc


def _rows(tt, c):
    """Global rows of core c's tt-th 128-token block (one 128-row piece).

    (b, hf) = (tt//2, tt%2); the ReduceScatter over rows [1024*hf, 1024*(hf+1))
    of batch b gives core c rows 128*c of that range.
    """
    b, hf = tt // 2, tt % 2
    return [b * T + hf * 1024 + 128 * c]


def kernel(x, w_norm1, w_qkv, w_proj, w_norm2, w_gate, w_up, w_down, step=0):
    x = np.asarray(x, dtype=np.float32)
    w_norm1 = np.asarray(w_norm1, dtype=np.float32)
    w_qkv = np.asarray(w_qkv, dtype=np.float32)
    w_proj = np.asarray(w_proj, dtype=np.float32)
    w_norm2 = np.asarray(w_norm2, dtype=np.float32)
    w_gate = np.asarray(w_gate, dtype=np.float32)
    w_up = np.asarray(w_up, dtype=np.float32)
    w_down = np.asarray(w_down, dtype=np.float32)

    nc = _build()

    import ml_dtypes as _mld
    xt = np.ascontiguousarray(x.transpose(0, 2, 1)).astype(_mld.bfloat16)  # [B, D, T]
    xf = x.reshape(B * T, D)
    import ml_dtypes
    bf16 = ml_dtypes.bfloat16
    w1r = np.ascontiguousarray(w_norm1.reshape(KT, 128).T)    # [128, KT]
    wg_t = np.ascontiguousarray(w_gate.T).astype(bf16)        # [D, DFF]
    wu_t = np.ascontiguousarray(w_up.T).astype(bf16)          # [D, DFF]
    wd_t = np.ascontiguousarray(w_down.T).astype(bf16)        # [DFF, D]

    in_maps = []
    for c in range(NC):
        xs_c = np.concatenate(
            [xf[g : g + 128] for tt in range(4) for g in _rows(tt, c)], axis=0
        )
        in_maps.append({
            "xt": xt,
            "xs": np.ascontiguousarray(xs_c),
            "wq_t": np.ascontiguousarray(w_qkv[HD * c : HD * (c + 1)].T).astype(bf16),
            "wk_t": np.ascontiguousarray(w_qkv[D + HD * c : D + HD * (c + 1)].T).astype(bf16),
            "wv_t": np.ascontiguousarray(
                w_qkv[2 * D + HD * c : 2 * D + HD * (c + 1)].T
            ).astype(bf16),
            "wp_t": np.ascontiguousarray(w_proj[:, HD * c : HD * (c + 1)].T).astype(bf16),
            "w1r": w1r,
            "w2": w_norm2,
            "wg_t": wg_t,
            "wu_t": wu_t,
            "wd_t": wd_t,
        })

    global _last_in_maps
    _last_in_maps = in_maps
    res = run_bass_kernel_spmd(nc, in_maps, core_ids=list(range(NC)))

    x_out = np.empty((B * T, D), dtype=np.float32)
    k_cache = np.empty((B, H, T, DH), dtype=np.float32)
    v_cache = np.empty((B, H, T, DH), dtype=np.float32)
    for c in range(NC):
        out_c = res.results[c]
        for tt in range(4):
            g = _rows(tt, c)[0]
            x_out[g : g + 128] = out_c["xo"][128 * tt : 128 * (tt + 1)]
        kc = out_c["ko"].reshape(B, T, HPC, DH).transpose(0, 2, 1, 3)
        vc = out_c["vo"].reshape(B, T, HPC, DH).transpose(0, 2, 1, 3)
        k_cache[:, HPC * c : HPC * (c + 1)] = kc
        v_cache[:, HPC * c : HPC * (c + 1)] = vc

    return x_out.reshape(B, T, D), k_cache, v_cache
